# revision 1
# baseline (speedup 1.0000x reference)
"""Causal MHSA (B=2, S=2048, D=1024, H=16, RoPE) on 8 TRN2 NeuronCores.

Sharding: each core owns 2 heads x both batches (tensor parallel on heads).
Attention is computed in transposed ("scores^T") layout so no on-device
transposes of Q/K/attn are needed; softmax denominators ride as an extra
ones-column in V. An 8-rank AllToAll re-shards from head-parallel to
sequence-parallel, after which each core runs the full output projection
for its (batch, seq-slice) and emits a [512, 1024] slice of the output.

Matmul operands are bf16 (fp32 accumulation in PSUM); input/output tensors
stay fp32 at the DRAM boundary of the problem. Host-side prep (data
movement only): transposes of x/W, an evens/odds permutation of the Wq/Wk
rows so RoPE becomes block ops (the permutation cancels inside q.k), rope
cos/sin tables, causal mask tile, one-hot selector for denominator
broadcast.
"""

import os
import numpy as np

import concourse.bass as bass
import concourse.bacc as bacc
import concourse.mybir as mybir
import concourse.tile as tile
from concourse.bass_utils import run_bass_kernel_spmd

B, S, D, H, DK = 2, 2048, 1024, 16, 64
NCORES = 8
HL = 2            # heads per core
DLOC = HL * DK    # 128 local head dims
SC = 512          # chunk of q / moving free dim
NQC = S // SC     # 4 q-chunks
NKT = S // 128    # 16 k-tiles
NET = D // 128    # 8 e-tiles (contraction over embedding)
SSL = S // 4      # 512 seq slice per rank (within a batch)
ROPE_THETA = 10000.0

f32 = mybir.dt.float32
f32r = mybir.dt.float32r
bf16 = mybir.dt.bfloat16

LAST_EXEC_NS = {"ns": None}
_CACHE = {}


def _build_nc():
    nc = bacc.Bacc(
        "TRN2", target_bir_lowering=False, debug=False, num_devices=NCORES
    )

    xT = [
        nc.dram_tensor(f"xT{b}", [D, S], bf16, kind="ExternalInput").ap()
        for b in range(B)
    ]
    wq = nc.dram_tensor("wq", [D, DLOC], bf16, kind="ExternalInput").ap()
    wk = nc.dram_tensor("wk", [D, DLOC], bf16, kind="ExternalInput").ap()
    perm_d = nc.dram_tensor("perm128", [128, 128], f32r,
                            kind="ExternalInput").ap()
    wv = nc.dram_tensor("wv", [D, DLOC], bf16, kind="ExternalInput").ap()
    woT = nc.dram_tensor("woT", [D, D], bf16, kind="ExternalInput").ap()
    cos_d = nc.dram_tensor("cos_t", [128, S], f32, kind="ExternalInput").ap()
    sin_d = nc.dram_tensor("sin_t", [128, S], f32, kind="ExternalInput").ap()
    mask_d = nc.dram_tensor("mask512", [128, 512], bf16,
                            kind="ExternalInput").ap()
    sel_d = nc.dram_tensor("sel16", [16, 1024], bf16, kind="ExternalInput").ap()
    id_d = nc.dram_tensor("id128", [128, 128], bf16, kind="ExternalInput").ap()
    ones_d = nc.dram_tensor("ones32", [128, 32], bf16,
                            kind="ExternalInput").ap()
    out = nc.dram_tensor("out", [SSL, D], f32, kind="ExternalOutput").ap()

    with tile.TileContext(nc) as tc:
        _body(nc, tc, xT, wq, wk, perm_d, wv, woT, cos_d, sin_d, mask_d,
              sel_d, id_d, ones_d, out)

    nc.compile()
    return nc


def _body(nc, tc, xT, wq, wk, perm_d, wv, woT, cos_d, sin_d, mask_d,
          sel_d, id_d, ones_d, out):
    Exp = mybir.ActivationFunctionType.Exp
    from concourse.bass import _add_dep_helper

    with (
        tc.tile_pool(name="const", bufs=1) as cpool,
        tc.tile_pool(name="projc", bufs=1) as pcpool,
        tc.tile_pool(name="qkv", bufs=1) as qkvp,
        tc.tile_pool(name="attn_sb", bufs=1) as apool,
        tc.tile_pool(name="dram", bufs=1, space="DRAM") as dpool,
    ):
        # ---- constants ----
        mask_sb = cpool.tile([128, 512], bf16, tag="mask")
        id_sb = cpool.tile([128, 128], bf16, tag="id")
        nc.sync.dma_start(mask_sb[:], mask_d[:])
        nc.sync.dma_start(id_sb[:], id_d[:])
        # [128, 2, 128/256] views of the triangular/full mask blocks
        mask_v = mask_sb[:].rearrange("p (a x) -> p a x", a=2)

        cos_sb = pcpool.tile([128, S], f32, tag="cos")
        sin_sb = pcpool.tile([128, S], f32, tag="sin")
        nc.sync.dma_start(cos_sb[:], cos_d[:])
        nc.sync.dma_start(sin_sb[:], sin_d[:])
        perm_sb = cpool.tile([128, 128], f32r, tag="perm")
        nc.sync.dma_start(perm_sb[:], perm_d[:])
        wnames = [("q", wq), ("k", wk), ("v", wv)]
        w_sb = {}
        for nm, wa in wnames:
            for et in range(NET):
                t = pcpool.tile([128, DLOC], bf16, tag=f"w{nm}{et}",
                                name=f"w_{nm}_{et}")
                nc.sync.dma_start(t[:], wa[128 * et:128 * et + 128, :])
                w_sb[nm, et] = t

        # ---- persistent tiles (single-batch q/k/v, per-batch attn out) ----
        qt_sb = qkvp.tile([128, S], f32r, tag="qt", name="qt")
        kt_sb = qkvp.tile([128, S], f32r, tag="kt", name="kt")
        v_sb = qkvp.tile([128, NKT * 130], bf16, tag="v", name="v")
        ones_view = v_sb[:].rearrange(
            "p (kt h x) -> p kt h x", kt=NKT, h=HL)[:, :, :, 64:65]
        nc.sync.dma_start(ones_view, ones_d[:])
        at_sb = {}
        for b in range(B):
            for h in range(HL):
                at_sb[b, h] = apool.tile([65, S], bf16, tag=f"at{b}{h}",
                                         name=f"at{b}{h}")

        for b in range(B):
            # ============== projection + rope + V assembly ==============
            with tc.tile_pool(name="xt", bufs=8) as xtp:
                xts = []
                for et in range(NET):
                    t = xtp.tile([128, S], bf16, tag="xt", name=f"xt{b}_{et}")
                    nc.sync.dma_start(t[:], xT[b][128 * et:128 * et + 128, :])
                    xts.append(t)

                with (
                    tc.tile_pool(name="ptmp", bufs=3) as rtp,
                    tc.tile_pool(name="pp", bufs=2, space="PSUM") as ppp,
                ):
                    for c in range(NQC):
                        cs = slice(SC * c, SC * c + SC)
                        for nm, dst in (("q", qt_sb), ("k", kt_sb)):
                            p = ppp.tile([128, SC], f32, tag=f"pp{nm}",
                                         name=f"pp{nm}_{b}_{c}")
                            for et in range(NET):
                                nc.tensor.matmul(
                                    p[:], w_sb[nm, et][:], xts[et][:, cs],
                                    start=(et == 0), stop=(et == NET - 1),
                                )
                            # partner tile via 32-block-swap permutation mm
                            t1c = rtp.tile([128, SC], f32r, tag="t1c",
                                           name=f"t1c{b}{c}{nm}")
                            nc.vector.tensor_copy(t1c[:], p[:])
                            p2 = ppp.tile([128, SC], f32, tag=f"pp{nm}2",
                                          name=f"pp{nm}2_{b}_{c}")
                            nc.tensor.matmul(p2[:], perm_sb[:], t1c[:],
                                             start=True, stop=True)
                            # rope: out = cos*T1 + sin_signed*T2
                            ta = rtp.tile([128, SC], f32, tag="ra",
                                          name=f"ra{b}{c}{nm}")
                            tb = rtp.tile([128, SC], f32, tag="rb",
                                          name=f"rb{b}{c}{nm}")
                            nc.vector.tensor_mul(ta[:], cos_sb[:, cs], p[:])
                            nc.vector.tensor_mul(tb[:], sin_sb[:, cs], p2[:])
                            nc.vector.tensor_add(dst[:, cs], ta[:], tb[:])

                # V projection + transposes into [s, d] layout
                with (
                    tc.tile_pool(name="vtmp", bufs=2) as vtp,
                    tc.tile_pool(name="pv", bufs=2, space="PSUM") as pvp,
                    tc.tile_pool(name="ptr", bufs=2, space="PSUM") as ptrp,
                ):
                    for c in range(NQC):
                        cs = slice(SC * c, SC * c + SC)
                        p = pvp.tile([128, SC], f32, tag="pv",
                                     name=f"pv_{b}_{c}")
                        for et in range(NET):
                            nc.tensor.matmul(
                                p[:], w_sb["v", et][:], xts[et][:, cs],
                                start=(et == 0), stop=(et == NET - 1),
                            )
                        vt_tmp = vtp.tile([128, SC], bf16, tag="vt",
                                          name=f"vt{b}{c}")
                        nc.vector.tensor_copy(vt_tmp[:], p[:])
                        for j in range(SC // 128):
                            ktile = (SC * c) // 128 + j
                            tp = ptrp.tile([128, 128], bf16, tag="tr",
                                           name=f"tr{b}{c}{j}")
                            nc.tensor.transpose(
                                tp[:], vt_tmp[:, 128 * j:128 * j + 128],
                                id_sb[:])
                            dst = v_sb[:, 130 * ktile:130 * ktile + 130]
                            dst = dst.rearrange("p (h x) -> p h x",
                                                h=HL)[:, :, 0:64]
                            nc.vector.tensor_copy(
                                dst, tp[:].rearrange("p (h x) -> p h x", h=HL))

            # ======================= attention =========================
            # scores for both heads share one [128, 1024] psum tile
            # (h0 cols 0:512, h1 cols 512:1024) -> single exp / mask op.
            with (
                tc.tile_pool(name="et", bufs=6) as etp,
                tc.tile_pool(name="psc", bufs=3, space="PSUM") as pscp,
                tc.tile_pool(name="pat", bufs=1, space="PSUM") as patp,
            ):
                for qc in range(NQC):
                    pa = {}
                    for h in range(HL):
                        pa[h] = patp.tile([65, SC], f32, tag=f"pa{h}",
                                          name=f"pa{b}{qc}{h}")
                    nkt = 4 * qc + 4
                    for kt in range(nkt):
                        rel = kt - 4 * qc
                        lo = 0 if rel < 0 else (128 * rel if rel < 3 else 256)
                        psc = pscp.tile([128, 2 * SC], f32, tag="ps",
                                        name=f"ps{b}{qc}{kt}")
                        for h in range(HL):
                            hs = slice(64 * h, 64 * h + 64)
                            nc.tensor.matmul(
                                psc[:, SC * h + lo:SC * h + SC],
                                kt_sb[hs, 128 * kt:128 * kt + 128],
                                qt_sb[hs, SC * qc + lo:SC * qc + SC],
                                start=True, stop=True,
                            )
                        et = etp.tile([128, 2 * SC], bf16, tag="et",
                                      name=f"et{b}{qc}{kt}")
                        psc_v = psc[:].rearrange("p (a x) -> p a x", a=2)
                        et_v = et[:].rearrange("p (a x) -> p a x", a=2)
                        nc.scalar.activation(et_v[:, :, lo:], psc_v[:, :, lo:],
                                             Exp)
                        if rel >= 0:
                            if rel < 3:
                                nc.vector.tensor_mul(
                                    et_v[:, :, lo:lo + 128],
                                    et_v[:, :, lo:lo + 128],
                                    mask_v[:, :, 128:256])
                            else:
                                nc.vector.tensor_mul(
                                    et_v[:, :, lo:lo + 256],
                                    et_v[:, :, lo:lo + 256],
                                    mask_v[:, :, 0:256])
                        for h in range(HL):
                            nc.tensor.matmul(
                                pa[h][:, lo:],
                                v_sb[:, 130 * kt + 65 * h:
                                     130 * kt + 65 * h + 65],
                                et[:, SC * h + lo:SC * h + SC],
                                start=(kt == 0), stop=(kt == nkt - 1),
                                skip_group_check=True,
                            )
                    for h in range(HL):
                        nc.vector.tensor_copy(
                            at_sb[b, h][:, SC * qc:SC * qc + SC], pa[h][:])

        # ================= all-to-all (head-parallel -> seq-parallel) ======
        a2a_in = dpool.tile([NCORES * 130, SSL], bf16, name="a2a_in")
        a2a_out = dpool.tile([NCORES * 130, SSL], bf16, name="a2a_out")
        for r_ in range(NCORES):
            br, sl = r_ // 4, r_ % 4
            for h in range(HL):
                if r_ == 7 and h == 1:
                    continue  # issued below so we can hang warmups off it
                nc.sync.dma_start(
                    a2a_in[130 * r_ + 65 * h:130 * r_ + 65 * h + 65, :],
                    at_sb[br, h][:, SSL * sl:SSL * sl + SSL])
        last_dma = nc.sync.dma_start(
            a2a_in[130 * 7 + 65:130 * 7 + 130, :],
            at_sb[1, 1][:, SSL * 3:SSL * 3 + SSL])
        nc.gpsimd.collective_compute(
            "AllToAll",
            mybir.AluOpType.bypass,
            replica_groups=[list(range(NCORES))],
            ins=[a2a_in.opt()],
            outs=[a2a_out.opt()],
        )

        # ================= normalize + output projection ==================
        with (
            tc.tile_pool(name="late", bufs=1) as lpool,
            tc.tile_pool(name="ocp", bufs=2) as ocp,
            tc.tile_pool(name="po", bufs=2, space="PSUM") as pop,
        ):
            wo_sb = []
            for i in range(NET):
                t = lpool.tile([128, D], bf16, tag=f"wo{i}", name=f"wo{i}")
                nc.sync.dma_start(t[:], woT[128 * i:128 * i + 128, :])
                wo_sb.append(t)

            sel_sb = lpool.tile([16, 1024], bf16, tag="sel", name="sel_sb")
            nc.sync.dma_start(sel_sb[:], sel_d[:])

            # keep the PE busy/warm across the collective
            wu = pop.tile([128, SC], f32, tag="wu", name="wu")
            first = None
            for wi in range(120):
                m = nc.tensor.matmul(wu[:], sel_sb[:, 0:128], sel_sb[:, 0:512],
                                     start=True, stop=True)
                if first is None:
                    first = m
            _add_dep_helper(first.ins, last_dma.ins, sync=True,
                            reason="pe warmup during collective")
            wu_s = ocp.tile([1, 4], f32, tag="wus", name="wu_s")
            nc.vector.tensor_copy(wu_s[:], wu[0:1, 0:4])
            wu_sink = dpool.tile([1, 4], f32, name="wu_sink")
            nc.sync.dma_start(wu_sink[:], wu_s[:])


            denom = lpool.tile([16, SSL], bf16, tag="den", name="denom")
            nc.sync.dma_start(
                denom[:],
                a2a_out[:].rearrange("(r h d) q -> r h d q", r=NCORES,
                                     h=HL)[:, :, 64:65, :])
            recip = lpool.tile([16, SSL], bf16, tag="rec", name="recip")
            with nc.allow_low_precision(reason="bf16 matmul input"):
                nc.vector.reciprocal(recip[:], denom[:])

            norm = []
            for i in range(NET):
                g = lpool.tile([128, SSL], bf16, tag=f"g{i}", name=f"g{i}")
                nc.sync.dma_start(
                    g[:],
                    a2a_out[130 * i:130 * i + 130, :].rearrange(
                        "(h x) q -> h x q", h=HL)[:, 0:64, :])
                pb = pop.tile([128, SSL], f32, tag="pb", name=f"pb{i}")
                nc.tensor.matmul(pb[:], sel_sb[:, 128 * i:128 * i + 128],
                                 recip[:], start=True, stop=True)
                nv = lpool.tile([128, SSL], bf16, tag=f"n{i}", name=f"n{i}")
                nc.vector.tensor_mul(nv[:], g[:], pb[:])
                norm.append(nv)

            for ec in range(2):
                for st in range(SSL // 128):
                    po = pop.tile([128, SC], f32, tag="po",
                                  name=f"po{ec}{st}")
                    for i in range(NET):
                        nc.tensor.matmul(
                            po[:],
                            norm[i][:, 128 * st:128 * st + 128],
                            wo_sb[i][:, SC * ec:SC * ec + SC],
                            start=(i == 0), stop=(i == NET - 1),
                        )
                    oc = ocp.tile([128, SC], f32, tag="oc",
                                  name=f"oc{ec}{st}")
                    nc.vector.tensor_copy(oc[:], po[:])
                    nc.sync.dma_start(
                        out[128 * st:128 * st + 128, SC * ec:SC * ec + SC],
                        oc[:])


def _host_prep(x, Wq, Wk, Wv, Wo):
    import ml_dtypes
    bf = ml_dtypes.bfloat16

    x = np.asarray(x, np.float32)
    Wq = np.asarray(Wq, np.float32)
    Wk = np.asarray(Wk, np.float32)
    Wv = np.asarray(Wv, np.float32)
    Wo = np.asarray(Wo, np.float32)

    perm = np.concatenate([np.arange(0, DK, 2), np.arange(1, DK, 2)])
    swap = np.concatenate([np.arange(32, 64), np.arange(0, 32)])
    swap128 = np.concatenate([swap, 64 + swap])

    freqs = 1.0 / (ROPE_THETA ** (np.arange(0, DK, 2, dtype=np.float64) / DK))
    ang = np.arange(S, dtype=np.float64)[:, None] * freqs[None, :]  # [S, 32]
    cos32 = np.cos(ang).T.astype(np.float32)  # [32, S]
    sin32 = np.sin(ang).T.astype(np.float32)
    cos_t = np.ascontiguousarray(np.tile(cos32, (4, 1)))
    sin_t = np.ascontiguousarray(
        np.concatenate([-sin32, sin32, -sin32, sin32], axis=0))

    tri = np.triu(np.ones((128, 128), np.float32))
    zero = np.zeros((128, 128), np.float32)
    mask512 = np.ascontiguousarray(
        np.concatenate([zero, tri, zero, tri], axis=1)).astype(bf)

    sel = np.zeros((16, 1024), np.float32)
    for i in range(NET):
        for m in range(128):
            sel[2 * i + m // 64, 128 * i + m] = 1.0

    id128 = np.eye(128, dtype=np.float32).astype(bf)
    permM = np.zeros((128, 128), np.float32)
    for r_ in range(128):
        permM[swap128[r_], r_] = 1.0

    xTb = [np.ascontiguousarray(x[b].T).astype(bf) for b in range(B)]

    scale = 1.0 / np.sqrt(np.float32(DK))
    in_maps = []
    for c in range(NCORES):
        rows = np.concatenate(
            [DK * (2 * c) + perm, DK * (2 * c + 1) + perm])
        wq_c = np.ascontiguousarray((scale * Wq[rows, :]).T)
        wk_c = np.ascontiguousarray(Wk[rows, :].T)
        wv_c = np.ascontiguousarray(
            Wv[DLOC * c:DLOC * c + DLOC, :].T).astype(bf)
        in_maps.append({
            "xT0": xTb[0], "xT1": xTb[1],
            "wq": wq_c.astype(bf),
            "wk": wk_c.astype(bf),
            "wv": wv_c,
            "perm128": permM,
            "woT": np.ascontiguousarray(Wo.T).astype(bf),
            "cos_t": cos_t, "sin_t": sin_t,
            "mask512": mask512, "sel16": sel.astype(bf), "id128": id128,
            "ones32": np.ones((128, 32), np.float32).astype(bf),
        })
    return in_maps


def _assemble(results):
    full = np.empty((B, S, D), np.float32)
    for r_ in range(NCORES):
        full[r_ // 4, SSL * (r_ % 4):SSL * (r_ % 4) + SSL, :] = \
            results[r_]["out"]
    return full


def kernel(x, Wq, Wk, Wv, Wo):
    if "nc" not in _CACHE:
        _CACHE["nc"] = _build_nc()
    nc = _CACHE["nc"]
    in_maps = _host_prep(x, Wq, Wk, Wv, Wo)

    if os.environ.get("MHA_SIM"):
        from concourse.bass_interp import MultiCoreSim
        sim = MultiCoreSim(nc, num_cores=NCORES)
        for c in range(NCORES):
            for k, v in in_maps[c].items():
                sim.cores[c].tensor(k)[:] = v
        sim.simulate()
        results = [{"out": np.array(sim.cores[c].mem_tensor("out"))}
                   for c in range(NCORES)]
        return _assemble(results)

    trace = bool(os.environ.get("MHA_TRACE"))
    res = run_bass_kernel_spmd(
        nc, in_maps, list(range(NCORES)), trace=trace)
    LAST_EXEC_NS["ns"] = res.exec_time_ns
    return _assemble(res.results)



# revision 4
# speedup vs baseline: 1.1033x; 1.1033x over previous
"""Causal MHSA (B=2, S=2048, D=1024, H=16, RoPE) on 8 TRN2 NeuronCores.

Sharding: each core owns 2 heads x both batches (tensor parallel on
heads). Attention runs in transposed ("scores^T") layout so no on-device
transposes of Q/K/attn are needed; softmax denominators ride as an extra
ones-column in V. An 8-rank AllToAll re-shards from head-parallel to
sequence-parallel, after which each core runs the output projection for
its (batch, seq-slice) and emits a [512, 1024] output slice.

All matmul operands are bf16 (fp32 accumulation in PSUM); q/k stay bf16
through RoPE so the per-head score matmuls run row-grouped and
concurrently at full bf16 rate. V is projected directly into [seq, dim]
layout (x s-chunks as the stationary operand), removing PE transposes.
Projection chunks and attention q-chunks are interleaved in program
order so the PE has dense work while the scalar engine (exp) grinds
through softmax; attention outputs are staged into the AllToAll input
eagerly per (batch, q-chunk) so the collective triggers immediately
after the last chunk. Input DMAs are split across the Sync/Scalar/GpSimd
queues and ordered so the first projection chunk starts early.
"""

import os
import numpy as np

import concourse.bass as bass
import concourse.bacc as bacc
import concourse.mybir as mybir
import concourse.tile as tile
from concourse.bass_utils import run_bass_kernel_spmd

B, S, D, H, DK = 2, 2048, 1024, 16, 64
NCORES = 8
HL = 2            # heads per core
DLOC = HL * DK    # 128 local head dims
SC = 512          # q-chunk / moving free dim
NQC = S // SC     # 4 q-chunks
NKT = S // 128    # 16 k-tiles
NET = D // 128    # 8 e-tiles (contraction over embedding)
SSL = S // 4      # 512-row seq slice per rank (within a batch)
VROW = 65         # 64 dims + denominator ones-row
ROPE_THETA = 10000.0

f32 = mybir.dt.float32
bf16 = mybir.dt.bfloat16

LAST_EXEC_NS = {"ns": None}
_CACHE = {}


def _build_nc():
    nc = bacc.Bacc(
        "TRN2", target_bir_lowering=False, debug=False, num_devices=NCORES
    )

    xT = [
        nc.dram_tensor(f"xT{b}", [D, S], bf16, kind="ExternalInput").ap()
        for b in range(B)
    ]
    wq = nc.dram_tensor("wq", [D, DLOC], bf16, kind="ExternalInput").ap()
    wk = nc.dram_tensor("wk", [D, DLOC], bf16, kind="ExternalInput").ap()
    wv = nc.dram_tensor("wv", [D, DLOC], bf16, kind="ExternalInput").ap()
    woT = nc.dram_tensor("woT", [D, D], bf16, kind="ExternalInput").ap()
    cos_d = nc.dram_tensor("cos_t", [128, S], bf16, kind="ExternalInput").ap()
    sin_d = nc.dram_tensor("sin_t", [128, S], bf16, kind="ExternalInput").ap()
    perm_d = nc.dram_tensor("perm128", [128, 128], bf16,
                            kind="ExternalInput").ap()
    mask_d = nc.dram_tensor("mask256", [128, 256], bf16,
                            kind="ExternalInput").ap()
    sel_d = nc.dram_tensor("sel16", [16, 1024], bf16, kind="ExternalInput").ap()
    out = nc.dram_tensor("out", [SSL, D], f32, kind="ExternalOutput").ap()

    with tile.TileContext(nc) as tc:
        _body(nc, tc, xT, wq, wk, wv, woT, cos_d, sin_d, perm_d, mask_d,
              sel_d, out)

    nc.compile()
    return nc


def _body(nc, tc, xT, wq, wk, wv, woT, cos_d, sin_d, perm_d, mask_d,
          sel_d, out):
    Exp = mybir.ActivationFunctionType.Exp

    with (
        tc.tile_pool(name="const", bufs=1) as cpool,
        tc.tile_pool(name="xw", bufs=1) as xwpool,
        tc.tile_pool(name="qkv", bufs=1) as qkvp,
        tc.tile_pool(name="dram", bufs=1, space="DRAM") as dpool,
    ):
        # ---------------- input DMAs, spread across queues ----------------
        # scalar queue: q/k weights first (needed by proj chunk 0), then
        # rope tables, then v weights.
        w_sb = {}
        for nm, wa in (("q", wq), ("k", wk)):
            for et in range(NET):
                t = cpool.tile([128, DLOC], bf16, tag=f"w{nm}{et}",
                               name=f"w_{nm}_{et}")
                nc.scalar.dma_start(t[:], wa[128 * et:128 * et + 128, :])
                w_sb[nm, et] = t
        cos_sb = cpool.tile([128, S], bf16, tag="cos")
        sin_sb = cpool.tile([128, S], bf16, tag="sin")
        nc.scalar.dma_start(cos_sb[:], cos_d[:])
        nc.scalar.dma_start(sin_sb[:], sin_d[:])
        for et in range(NET):
            t = cpool.tile([128, DLOC], bf16, tag=f"wv{et}", name=f"w_v_{et}")
            nc.scalar.dma_start(t[:], wv[128 * et:128 * et + 128, :])
            w_sb["v", et] = t

        # sync queue: x chunks, batch 0 chunk-major first, then batch 1.
        xts = {}
        for b in range(B):
            for et in range(NET):
                t = xwpool.tile([128, S], bf16, tag=f"xt{b}{et}",
                                name=f"xt{b}_{et}")
                xts[b, et] = t
        for b in range(B):
            for c in range(NQC):
                cs = slice(SC * c, SC * c + SC)
                for et in range(NET):
                    nc.sync.dma_start(xts[b, et][:, cs],
                                      xT[b][128 * et:128 * et + 128, cs])

        # gpsimd queue: everything not needed before attention/outproj.
        perm_sb = cpool.tile([128, 128], bf16, tag="perm")
        nc.gpsimd.dma_start(perm_sb[:], perm_d[:])
        mask_sb = cpool.tile([128, 256], bf16, tag="mask")
        nc.gpsimd.dma_start(mask_sb[:], mask_d[:])
        mask_v = mask_sb[:].rearrange("p (a x) -> p a x", a=2)
        sel_sb = cpool.tile([16, 1024], bf16, tag="sel")
        nc.gpsimd.dma_start(sel_sb[:], sel_d[:])
        wo_sb = []
        for i in range(NET):
            t = cpool.tile([128, D], bf16, tag=f"wo{i}", name=f"wo{i}")
            nc.gpsimd.dma_start(t[:], woT[128 * i:128 * i + 128, :])
            wo_sb.append(t)

        # ---------------- persistent q/k/v tiles (per batch) --------------
        qt_sb = [qkvp.tile([128, S], bf16, tag=f"qt{b}", name=f"qt{b}")
                 for b in range(B)]
        kt_sb = [qkvp.tile([128, S], bf16, tag=f"kt{b}", name=f"kt{b}")
                 for b in range(B)]
        # v in [seq, head, dim] layout: [128, kt, 2 heads, 65]
        v_sb = [qkvp.tile([128, NKT * HL * VROW], bf16, tag=f"v{b}",
                          name=f"v{b}") for b in range(B)]
        v_view = [v_sb[b][:].rearrange("p (kt h x) -> p kt h x",
                                       kt=NKT, h=HL) for b in range(B)]
        for b in range(B):
            nc.vector.memset(v_view[b][:, :, :, 64:65], 1.0)

        a2a_in = dpool.tile([NCORES * HL * VROW, SSL], bf16, name="a2a_in")
        a2a_out = dpool.tile([NCORES * HL * VROW, SSL], bf16, name="a2a_out")

        with (
            tc.tile_pool(name="pp", bufs=2, space="PSUM") as ppp,
            tc.tile_pool(name="psc", bufs=2, space="PSUM") as pscp,
            tc.tile_pool(name="pat", bufs=1, space="PSUM") as patp,
            tc.tile_pool(name="rtmp", bufs=2) as rtp,
            tc.tile_pool(name="et", bufs=4) as etp,
            tc.tile_pool(name="stg", bufs=2) as stgp,
        ):
            def proj_chunk(b, c):
                cs = slice(SC * c, SC * c + SC)
                for nm, dst in (("q", qt_sb[b]), ("k", kt_sb[b])):
                    p = ppp.tile([128, SC], f32, tag="pp",
                                 name=f"pp{nm}_{b}_{c}")
                    for et in range(NET):
                        nc.tensor.matmul(
                            p[:], w_sb[nm, et][:], xts[b, et][:, cs],
                            start=(et == 0), stop=(et == NET - 1),
                        )
                    t1c = rtp.tile([128, SC], bf16, tag="t1c",
                                   name=f"t1c{b}{c}{nm}")
                    nc.vector.tensor_copy(t1c[:], p[:])
                    p2 = ppp.tile([128, SC], f32, tag="pp",
                                  name=f"pp2{nm}_{b}_{c}")
                    nc.tensor.matmul(p2[:], perm_sb[:], t1c[:],
                                     start=True, stop=True)
                    ta = rtp.tile([128, SC], f32, tag="ra",
                                  name=f"ra{b}{c}{nm}")
                    tb = rtp.tile([128, SC], f32, tag="rb",
                                  name=f"rb{b}{c}{nm}")
                    nc.vector.tensor_mul(ta[:], cos_sb[:, cs], p[:])
                    nc.vector.tensor_mul(tb[:], sin_sb[:, cs], p2[:])
                    nc.vector.tensor_add(dst[:, cs], ta[:], tb[:])
                # v straight into [s, d] layout: stationary = x s-slices
                for j in range(SC // 128):
                    kt = (SC * c) // 128 + j
                    ss = slice(128 * kt, 128 * kt + 128)
                    pv = ppp.tile([128, SC], f32, tag="pp",
                                  name=f"ppv_{b}_{c}_{j}")
                    for et in range(NET):
                        nc.tensor.matmul(
                            pv[:, 0:DLOC], xts[b, et][:, ss],
                            w_sb["v", et][:],
                            start=(et == 0), stop=(et == NET - 1),
                        )
                    nc.vector.tensor_copy(
                        v_view[b][:, kt, :, 0:64],
                        pv[:, 0:DLOC].rearrange("p (h x) -> p h x", h=HL))

            def attn_chunk(b, qc):
                pa = {}
                for h in range(HL):
                    pa[h] = patp.tile([VROW, SC], f32, tag=f"pa{h}",
                                      name=f"pa{b}{qc}{h}")
                nkt = 4 * qc + 4
                for kt in range(nkt):
                    rel = kt - 4 * qc
                    lo = 0 if rel < 0 else 128 * rel
                    psc = pscp.tile([128, 2 * SC], f32, tag="ps",
                                    name=f"ps{b}{qc}{kt}")
                    for h in range(HL):
                        hs = slice(64 * h, 64 * h + 64)
                        nc.tensor.matmul(
                            psc[:, SC * h + lo:SC * h + SC],
                            kt_sb[b][hs, 128 * kt:128 * kt + 128],
                            qt_sb[b][hs, SC * qc + lo:SC * qc + SC],
                            start=True, stop=True,
                        )
                    et = etp.tile([128, 2 * SC], bf16, tag="et",
                                  name=f"et{b}{qc}{kt}")
                    psc_v = psc[:].rearrange("p (a x) -> p a x", a=2)
                    et_v = et[:].rearrange("p (a x) -> p a x", a=2)
                    nc.scalar.activation(et_v[:, :, lo:], psc_v[:, :, lo:],
                                         Exp)
                    if rel >= 0:
                        nc.vector.tensor_mul(
                            et_v[:, :, lo:lo + 128],
                            et_v[:, :, lo:lo + 128],
                            mask_v[:, :, 0:128])
                    for h in range(HL):
                        nc.tensor.matmul(
                            pa[h][:, lo:],
                            v_view[b][:, kt, h, :],
                            et[:, SC * h + lo:SC * h + SC],
                            start=(kt == 0), stop=(kt == nkt - 1),
                            skip_group_check=True,
                        )
                # stage this (b, qc) block straight into the a2a input:
                # dst rank r = 4*b + qc gets rows [130r, 130r+130).
                stg = stgp.tile([VROW, 2 * SC], bf16, tag="stg",
                                name=f"stg{b}{qc}")
                for h in range(HL):
                    nc.vector.tensor_copy(stg[:, SC * h:SC * h + SC],
                                          pa[h][:])
                r = 4 * b + qc
                dst = a2a_in[HL * VROW * r:HL * VROW * r + HL * VROW, :]
                nc.sync.dma_start(
                    dst.rearrange("(a p) x -> p a x", a=2),
                    stg[:].rearrange("p (a x) -> p a x", a=2))

            # ---------------- interleaved schedule ----------------
            proj_chunk(0, 0)
            proj_chunk(0, 1)
            attn_chunk(0, 0)
            proj_chunk(0, 2)
            attn_chunk(0, 1)
            proj_chunk(0, 3)
            attn_chunk(0, 2)
            proj_chunk(1, 0)
            attn_chunk(0, 3)
            proj_chunk(1, 1)
            attn_chunk(1, 0)
            proj_chunk(1, 2)
            attn_chunk(1, 1)
            proj_chunk(1, 3)
            attn_chunk(1, 2)
            attn_chunk(1, 3)

        nc.gpsimd.collective_compute(
            "AllToAll",
            mybir.AluOpType.bypass,
            replica_groups=[list(range(NCORES))],
            ins=[a2a_in.opt()],
            outs=[a2a_out.opt()],
        )

        # ---------------- normalize + output projection ----------------
        with (
            tc.tile_pool(name="late", bufs=1) as lpool,
            tc.tile_pool(name="ocp", bufs=2) as ocp,
            tc.tile_pool(name="po", bufs=2, space="PSUM") as pop,
            tc.tile_pool(name="pb", bufs=2, space="PSUM") as pbp,
        ):
            # a2a_out rows: [src rank r (8) x head h (2) x VROW], cols SSL.
            a2a_v = a2a_out[:].rearrange("(r h p) q -> r h p q", r=NCORES,
                                         h=HL)
            denom = lpool.tile([16, SSL], bf16, tag="den", name="denom")
            nc.scalar.dma_start(denom[:], a2a_v[:, :, 64:65, :])
            recip = lpool.tile([16, SSL], bf16, tag="rec", name="recip")
            with nc.allow_low_precision(reason="bf16 matmul input"):
                nc.vector.reciprocal(recip[:], denom[:])

            norm = []
            for i in range(NET):
                g = lpool.tile([128, SSL], bf16, tag=f"g{i}", name=f"g{i}")
                # e-tile i = global heads 2i, 2i+1 = src rank i, h 0..1
                nc.scalar.dma_start(g[:], a2a_v[i, :, 0:64, :])
                pb = pbp.tile([128, SSL], f32, tag="pb", name=f"pb{i}")
                nc.tensor.matmul(pb[:], sel_sb[:, 128 * i:128 * i + 128],
                                 recip[:], start=True, stop=True)
                nv = lpool.tile([128, SSL], bf16, tag=f"n{i}", name=f"n{i}")
                nc.vector.tensor_mul(nv[:], g[:], pb[:])
                norm.append(nv)

            for ec in range(2):
                for st in range(SSL // 128):
                    po = pop.tile([128, SC], f32, tag="po",
                                  name=f"po{ec}{st}")
                    for i in range(NET):
                        nc.tensor.matmul(
                            po[:],
                            norm[i][:, 128 * st:128 * st + 128],
                            wo_sb[i][:, SC * ec:SC * ec + SC],
                            start=(i == 0), stop=(i == NET - 1),
                        )
                    oc = ocp.tile([128, SC], f32, tag="oc",
                                  name=f"oc{ec}{st}")
                    nc.vector.tensor_copy(oc[:], po[:])
                    nc.sync.dma_start(
                        out[128 * st:128 * st + 128, SC * ec:SC * ec + SC],
                        oc[:])


def _host_prep(x, Wq, Wk, Wv, Wo):
    import ml_dtypes
    bf = ml_dtypes.bfloat16

    x = np.asarray(x, np.float32)
    Wq = np.asarray(Wq, np.float32)
    Wk = np.asarray(Wk, np.float32)
    Wv = np.asarray(Wv, np.float32)
    Wo = np.asarray(Wo, np.float32)

    perm = np.concatenate([np.arange(0, DK, 2), np.arange(1, DK, 2)])
    swap = np.concatenate([np.arange(32, 64), np.arange(0, 32)])
    swap128 = np.concatenate([swap, 64 + swap])

    freqs = 1.0 / (ROPE_THETA ** (np.arange(0, DK, 2, dtype=np.float64) / DK))
    ang = np.arange(S, dtype=np.float64)[:, None] * freqs[None, :]  # [S, 32]
    cos32 = np.cos(ang).T.astype(np.float32)  # [32, S]
    sin32 = np.sin(ang).T.astype(np.float32)
    cos_t = np.ascontiguousarray(np.tile(cos32, (4, 1))).astype(bf)
    sin_t = np.ascontiguousarray(
        np.concatenate([-sin32, sin32, -sin32, sin32], axis=0)).astype(bf)

    tri = np.triu(np.ones((128, 128), np.float32))
    mask256 = np.ascontiguousarray(
        np.concatenate([tri, tri], axis=1)).astype(bf)

    sel = np.zeros((16, 1024), np.float32)
    for i in range(NET):
        for m in range(128):
            sel[2 * i + m // 64, 128 * i + m] = 1.0

    permM = np.zeros((128, 128), np.float32)
    for r_ in range(128):
        permM[swap128[r_], r_] = 1.0

    xTb = [np.ascontiguousarray(x[b].T).astype(bf) for b in range(B)]
    woT = np.ascontiguousarray(Wo.T).astype(bf)

    scale = 1.0 / np.sqrt(np.float32(DK))
    in_maps = []
    for c in range(NCORES):
        rows = np.concatenate(
            [DK * (2 * c) + perm, DK * (2 * c + 1) + perm])
        wq_c = np.ascontiguousarray((scale * Wq[rows, :]).T)
        wk_c = np.ascontiguousarray(Wk[rows, :].T)
        wv_c = np.ascontiguousarray(
            Wv[DLOC * c:DLOC * c + DLOC, :].T).astype(bf)
        in_maps.append({
            "xT0": xTb[0], "xT1": xTb[1],
            "wq": wq_c.astype(bf),
            "wk": wk_c.astype(bf),
            "wv": wv_c,
            "woT": woT,
            "cos_t": cos_t, "sin_t": sin_t,
            "perm128": permM.astype(bf),
            "mask256": mask256, "sel16": sel.astype(bf),
        })
    return in_maps


def _assemble(results):
    full = np.empty((B, S, D), np.float32)
    for r_ in range(NCORES):
        full[r_ // 4, SSL * (r_ % 4):SSL * (r_ % 4) + SSL, :] = \
            results[r_]["out"]
    return full


def kernel(x, Wq, Wk, Wv, Wo):
    if "nc" not in _CACHE:
        _CACHE["nc"] = _build_nc()
    nc = _CACHE["nc"]
    in_maps = _host_prep(x, Wq, Wk, Wv, Wo)

    if os.environ.get("MHA_SIM"):
        from concourse.bass_interp import MultiCoreSim
        sim = MultiCoreSim(nc, num_cores=NCORES)
        for c in range(NCORES):
            for k, v in in_maps[c].items():
                sim.cores[c].tensor(k)[:] = v
        sim.simulate()
        results = [{"out": np.array(sim.cores[c].mem_tensor("out"))}
                   for c in range(NCORES)]
        return _assemble(results)

    trace = bool(os.environ.get("MHA_TRACE"))
    res = run_bass_kernel_spmd(
        nc, in_maps, list(range(NCORES)), trace=trace)
    LAST_EXEC_NS["ns"] = res.exec_time_ns
    return _assemble(res.results)
